# revision 17
# baseline (speedup 1.0000x reference)
"""Trainium2 Bass kernel for nn_Domain_adaptation (LMMD + discriminator/classifier losses).

Strategy (8 NeuronCores, feature-parallel):
  - The feature dim F=32768 is sharded 8x (4096 per core). Each core holds the
    transposed bf16 slices xT/yT = [4096, 1280] of source/target features.
  - Each core computes partial Gram matrices (src x tgt cross-Gram for Kxy,
    10 per-region self-Grams each for Kx / Ky), partial first-layer MLP outputs
    (dom @ dW1, tgt @ cW1), and partial squared-norm rows, all over its feature
    slice, on the PE array in bf16 (products of bf16 are exact in f32 PSUM).
  - A rank-1 term  -0.5 * ||col||^2  is folded into every Gram accumulation, so
    post-collective tiles hold  arg_pq = dot(p,q) - 0.5*||q||^2  directly.
  - Collectives (on-chip, overlap with compute):
      RS-A : ReduceScatter cross-Gram   [1280,1280] bf16 -> [160,1280]/core
      RS-B1: ReduceScatter h1T blocks   [1024, 320] f32  -> [128,320]/core
      RS-B2: ReduceScatter c1T blocks   [1024, 160] f32  -> [128,160]/core
      RS-B3: ReduceScatter x2 row       [1280, 1]  f32  -> [160,1]/core
      AR-C : AllReduce Kx/Ky self-Grams [2560, 128] bf16 (all cores get all)
  - exp(-d/2s^2): ACT engine, arg = G'_pq + bias_p with bias_p = -0.5*||p||^2.
    For Kx/Ky the bias is re-extracted from the *summed* Gram's own diagonal, so
    the diagonal argument is exactly 0 -> exp = 1 exactly (off-diagonals are
    ~-30000 and underflow to 0, as in the reference).
  - Each core finishes its batch slice of the discriminator/classifier MLPs
    (tiny f32 matmuls) and emits preds rows plus per-partition partial sums.
  - Host: gathers preds, sums partial scalars, computes log-softmax NLL, the L2
    term, and assembles the five outputs.
"""

import numpy as np
import ml_dtypes

BF16 = ml_dtypes.bfloat16

N = 1280          # batch (source and target)
F = 32768         # feature dim
NCORES = 8
FS = F // NCORES  # 4096 features per core
KCH = FS // 128   # 32 contraction chunks of 128
R = 10            # regions
NLOC = N // R     # 128 rows per region
L2_LAMBDA = 0.01
# 1/(2*sigma^2) with sigma=1 -> exp(-(d)*0.5); we compute arg = -0.5*d directly.

_CACHE = {}


def _build_program():
    import concourse.bass as bass
    import concourse.mybir as mybir
    import concourse.tile as tile
    from concourse import bacc
    from concourse.masks import make_identity

    fp32 = mybir.dt.float32
    bf16 = mybir.dt.bfloat16

    nc = bacc.Bacc(None, target_bir_lowering=False)

    # ---------------- I/O ----------------
    xt = nc.dram_tensor("xt", [FS, N], bf16, kind="ExternalInput")
    yt = nc.dram_tensor("yt", [FS, N], bf16, kind="ExternalInput")
    w1 = nc.dram_tensor("w1", [FS, 128], bf16, kind="ExternalInput")    # dW1 shard
    cw1 = nc.dram_tensor("cw1", [FS, 128], bf16, kind="ExternalInput")  # cW1 shard
    # packed small params [128, 107] f32: see _prep_inputs for column map
    params = nc.dram_tensor("params", [128, 107], fp32, kind="ExternalInput")

    predsT_o = nc.dram_tensor("predsT", [3, N // NCORES], fp32, kind="ExternalOutput")
    accs_o = nc.dram_tensor("accs", [128, 24], fp32, kind="ExternalOutput")

    rg = [list(range(NCORES))]
    HB = N // NCORES // 2  # 80: half-chunk rows of the split cross-Gram RS

    with tile.TileContext(nc) as tc:
        with (
            tc.tile_pool(name="small", bufs=1) as small,
            tc.tile_pool(name="dram", bufs=1, space="DRAM") as dram,
        ):
            # ------------- internal DRAM (collective bounce buffers) -------------
            rsa_inA = dram.tile([N // 2, N], bf16, tag="rsa_inA", name="rsa_inA")
            rsa_inB = dram.tile([N // 2, N], bf16, tag="rsa_inB", name="rsa_inB")
            rsa_outA = dram.tile([HB, N], bf16, tag="rsa_outA", name="rsa_outA")
            rsa_outB = dram.tile([HB, N], bf16, tag="rsa_outB", name="rsa_outB")
            rsb1_in = dram.tile([NCORES * 128, 320], bf16, tag="rsb1_in", name="rsb1_in")
            rsb1_out = dram.tile([128, 320], bf16, tag="rsb1_out", name="rsb1_out")
            rsb2_in = dram.tile([NCORES * 128, 160], fp32, tag="rsb2_in", name="rsb2_in")
            rsb2_out = dram.tile([128, 160], fp32, tag="rsb2_out", name="rsb2_out")
            rsb3_in = dram.tile([NCORES, 2, N // 2 // NCORES], bf16, tag="rsb3_in", name="rsb3_in")
            rsb3_out = dram.tile([2, N // 2 // NCORES], bf16, tag="rsb3_out", name="rsb3_out")
            arc_in = dram.tile([2 * R * 128, 128], bf16, tag="arc_in", name="arc_in")
            arc_out = dram.tile([2 * R * 128, 128], bf16, addr_space="Shared", tag="arc_out", name="arc_out")

            # ------------- persistent small tiles -------------
            pars = small.tile([128, 107], fp32, tag="pars", name="pars")
            nc.sync.dma_start(pars[:], params[:])
            # column map (host keeps in sync): db1 0; dw2 1:65; db2 65; dw3 66:68;
            # db3 68; cb1 69; cw2 70:102; cb2 102; cw3 103:106; cb3 106
            db1s = pars[:, 0:1]
            dw2s = pars[:, 1:65]
            db2s = pars[0:64, 65:66]
            dw3s = pars[0:64, 66:68]
            db3s = pars[0:2, 68:69]
            cb1s = pars[:, 69:70]
            cw2s = pars[:, 70:102]
            cb2s = pars[0:32, 102:103]
            cw3s = pars[0:32, 103:106]
            cb3s = pars[0:3, 106:107]

            ident = small.tile([128, 128], fp32, tag="ident", name="ident")
            make_identity(nc, ident[:])
            ones_col = small.tile([128, 1], fp32, tag="ones_col", name="ones_col")
            nc.gpsimd.memset(ones_col[:], 1.0)
            ones_row_b = small.tile([1, 128], bf16, tag="ones_row_b", name="ones_row_b")
            nc.gpsimd.memset(ones_row_b[:], 1.0)
            # -0.5*||col||^2 partial rows (bf16): [0:N]=src, [N:2N]=tgt
            xy2rowb = small.tile([1, 2 * N], bf16, tag="xy2rowb", name="xy2rowb")
            accs = small.tile([128, 24], fp32, tag="accs", name="accs")
            nc.gpsimd.memset(accs[:], 0.0)

            # ================= phase 1: load + MLP layer-1 partials =================
            ctx_main = tc.tile_pool(name="data", bufs=1)
            data = ctx_main.__enter__()
            ctx_p1 = tc.tile_pool(name="p1", bufs=1)
            p1 = ctx_p1.__enter__()

            doms = []
            for k in range(KCH):
                d = data.tile([128, 2 * N], bf16, tag=f"dom{k}", name=f"dom{k}")
                nc.sync.dma_start(d[:, 0:N], xt[128 * k:128 * (k + 1), :])
                nc.sync.dma_start(d[:, N:2 * N], yt[128 * k:128 * (k + 1), :])
                doms.append(d)

            w1all = p1.tile([128, KCH * 128], bf16, tag="w1all", name="w1all")
            c1all = p1.tile([128, KCH * 128], bf16, tag="c1all", name="c1all")
            for k in range(KCH):
                nc.sync.dma_start(w1all[:, 128 * k:128 * (k + 1)], w1[128 * k:128 * (k + 1), :])
                nc.sync.dma_start(c1all[:, 128 * k:128 * (k + 1)], cw1[128 * k:128 * (k + 1), :])

            ctx_ps1 = tc.tile_pool(name="ps1", bufs=1, space="PSUM")
            ps1 = ctx_ps1.__enter__()
            h1ps = [ps1.tile([128, 512], fp32, tag=f"h1ps{b}", name=f"h1ps{b}") for b in range(5)]
            c1ps = [ps1.tile([128, 512], fp32, tag=f"c1ps{b}", name=f"c1ps{b}") for b in range(3)]
            for k in range(KCH):
                for b in range(5):
                    nc.tensor.matmul(
                        h1ps[b][:], w1all[:, 128 * k:128 * (k + 1)],
                        doms[k][:, 512 * b:512 * (b + 1)],
                        start=(k == 0), stop=(k == KCH - 1),
                    )
                for b in range(3):
                    w = 512 if b < 2 else 256
                    nc.tensor.matmul(
                        c1ps[b][:, 0:w], c1all[:, 128 * k:128 * (k + 1)],
                        doms[k][:, N + 512 * b:N + 512 * b + w],
                        start=(k == 0), stop=(k == KCH - 1),
                    )

            # raw partial spill staging (bias/relu happen post-collective)
            h1sb = p1.tile([128, 8, 320], bf16, tag="h1sb", name="h1sb")
            c1sb = p1.tile([128, 8, 160], fp32, tag="c1sb", name="c1sb")
            for c in range(8):
                lo, hi = 320 * c, 320 * (c + 1)
                b0, b1 = lo // 512, (hi - 1) // 512
                for b in range(b0, b1 + 1):
                    s, e = max(lo, 512 * b), min(hi, 512 * (b + 1))
                    nc.scalar.copy(h1sb[:, c, s - lo:e - lo], h1ps[b][:, s - 512 * b:e - 512 * b])
                lo, hi = 160 * c, 160 * (c + 1)
                b0, b1 = lo // 512, (hi - 1) // 512
                for b in range(b0, b1 + 1):
                    s, e = max(lo, 512 * b), min(hi, 512 * (b + 1))
                    nc.scalar.copy(c1sb[:, c, s - lo:e - lo], c1ps[b][:, s - 512 * b:e - 512 * b])

            nc.sync.dma_start(rsb1_in.rearrange("(c p) j -> p c j", c=8), h1sb[:])
            nc.sync.dma_start(rsb2_in.rearrange("(c p) j -> p c j", c=8), c1sb[:])
            nc.gpsimd.collective_compute(
                "ReduceScatter", mybir.AluOpType.add, replica_groups=rg,
                ins=[rsb1_in[:]], outs=[rsb1_out[:]],
            )
            nc.gpsimd.collective_compute(
                "ReduceScatter", mybir.AluOpType.add, replica_groups=rg,
                ins=[rsb2_in[:]], outs=[rsb2_out[:]],
            )
            ctx_ps1.__exit__(None, None, None)
            ctx_p1.__exit__(None, None, None)

            # ================= phase 2a: per-region self-Grams (Kx, Ky) =================
            ctx_p2 = tc.tile_pool(name="p2", bufs=1)
            p2 = ctx_p2.__enter__()
            ctx_ps2 = tc.tile_pool(name="ps2", bufs=1, space="PSUM")
            ps2 = ctx_ps2.__enter__()
            sgps = [ps2.tile([128, 512], fp32, tag=f"sg{b}", name=f"sg{b}") for b in range(5)]

            def sg_ap(g):
                return sgps[g // 4][:, 128 * (g % 4):128 * (g % 4 + 1)]

            for k in range(KCH):
                for g in range(2 * R):
                    col = doms[k][:, 128 * g:128 * (g + 1)]
                    nc.tensor.matmul(sg_ap(g), col, col, start=(k == 0), stop=False)

            # -0.5 * diag rows (squared norms) from the partial Grams
            for g in range(2 * R):
                masked = p2.tile([128, 128], fp32, tag="masked", name="masked", bufs=2)
                nc.vector.scalar_tensor_tensor(
                    masked[:], sg_ap(g), 1.0, ident[:],
                    mybir.AluOpType.mult, mybir.AluOpType.mult,
                )
                rowp = ps2.tile([1, 128], fp32, tag="rowp", name="rowp", bufs=2)
                nc.tensor.matmul(rowp[:], ones_col[:], masked[:], start=True, stop=True)
                nc.scalar.activation(
                    xy2rowb[0:1, 128 * g:128 * (g + 1)], rowp[:],
                    mybir.ActivationFunctionType.Copy, scale=-0.5,
                )

            # fold rank-1 (-0.5*||q||^2) into each self-Gram, then stage as bf16
            gsall = p2.tile([128, 2 * R, 128], bf16, tag="gsall", name="gsall")
            for g in range(2 * R):
                nc.tensor.matmul(
                    sg_ap(g), ones_row_b[:], xy2rowb[0:1, 128 * g:128 * (g + 1)],
                    start=False, stop=True,
                )
                nc.scalar.copy(gsall[:, g, :], sg_ap(g))

            nc.scalar.dma_start(arc_in.rearrange("(g p) j -> p g j", g=2 * R), gsall[:])
            # x2 halves, permuted so RS chunk c = [rows 80c:80c+80 | rows 640+80c:...]
            nc.scalar.dma_start(rsb3_in[:, 0, :], xy2rowb[0:1, 0:N // 2])
            nc.scalar.dma_start(rsb3_in[:, 1, :], xy2rowb[0:1, N // 2:N])
            nc.gpsimd.collective_compute(
                "AllReduce", mybir.AluOpType.add, replica_groups=rg,
                ins=[arc_in[:]], outs=[arc_out[:]],
            )
            nc.gpsimd.collective_compute(
                "ReduceScatter", mybir.AluOpType.add, replica_groups=rg,
                ins=[rsb3_in[:]], outs=[rsb3_out[:]],
            )
            ctx_ps2.__exit__(None, None, None)

            # ================= phase 2b: cross-Gram Kxy (two halves) =================
            ctx_ps3 = tc.tile_pool(name="ps3", bufs=2, space="PSUM")
            ps3 = ctx_ps3.__enter__()
            widths = (512, 512, 256)
            for half, rsa_in in ((0, rsa_inA), (1, rsa_inB)):
                gmball = p2.tile([128, 5, N], bf16, tag=f"gmball{half}", name=f"gmball{half}")
                for mi in range(5):
                    m = 5 * half + mi
                    kxy = [
                        ps3.tile([128, 512], fp32, tag="kxy0", name="kxy0"),
                        ps3.tile([128, 512], fp32, tag="kxy1", name="kxy1"),
                        ps3.tile([128, 512], fp32, tag="kxy2", name="kxy2"),
                    ]
                    for k in range(KCH):
                        lhs = doms[k][:, 128 * m:128 * (m + 1)]
                        for b in range(3):
                            w = widths[b]
                            nc.tensor.matmul(
                                kxy[b][:, 0:w], lhs,
                                doms[k][:, N + 512 * b:N + 512 * b + w],
                                start=(k == 0), stop=False,
                            )
                    for b in range(3):
                        w = widths[b]
                        nc.tensor.matmul(
                            kxy[b][:, 0:w], ones_row_b[:],
                            xy2rowb[0:1, N + 512 * b:N + 512 * b + w],
                            start=False, stop=True,
                        )
                        nc.scalar.copy(gmball[:, mi, 512 * b:512 * b + w], kxy[b][:, 0:w])
                nc.scalar.dma_start(rsa_in.rearrange("(m p) j -> p m j", m=5), gmball[:])
                nc.gpsimd.collective_compute(
                    "ReduceScatter", mybir.AluOpType.add, replica_groups=rg,
                    ins=[rsa_in[:]], outs=[rsa_outA[:] if half == 0 else rsa_outB[:]],
                )
            ctx_ps3.__exit__(None, None, None)
            ctx_p2.__exit__(None, None, None)
            ctx_main.__exit__(None, None, None)

            # ================= post phase =================
            ctx_post = tc.tile_pool(name="post", bufs=1)
            post = ctx_post.__enter__()
            ctx_ps4 = tc.tile_pool(name="ps4", bufs=1, space="PSUM")
            psmall = ctx_ps4.__enter__()

            # ---- Kx/Ky exp-sums (every core, all regions, identical result) ----
            ggall = post.tile([128, 2 * R, 128], bf16, tag="ggall", name="ggall")
            nc.sync.dma_start(ggall[:], arc_out.rearrange("(g p) j -> p g j", g=2 * R))
            for g in range(2 * R):
                masked2 = post.tile([128, 128], fp32, tag="masked2", name="masked2", bufs=2)
                nc.vector.scalar_tensor_tensor(
                    masked2[:], ggall[:, g, :], 1.0, ident[:],
                    mybir.AluOpType.mult, mybir.AluOpType.mult,
                )
                negd = post.tile([128, 1], fp32, tag="negd", name="negd", bufs=2)
                nc.vector.tensor_reduce(
                    negd[:], masked2[:], mybir.AxisListType.X,
                    mybir.AluOpType.add, negate=True,
                )
                ex = post.tile([128, 128], bf16, tag="ex", name="ex", bufs=2)
                nc.scalar.activation(
                    ex[:], ggall[:, g, :], mybir.ActivationFunctionType.Exp,
                    bias=negd[:], accum_out=accs[:, g:g + 1],
                )

            # ---- Kxy chunk exp-sums (my two 80-row half-chunks) ----
            x2c = post.tile([HB, 2], bf16, tag="x2c", name="x2c")
            nc.sync.dma_start(x2c[:, 0], rsb3_out[0, :])
            nc.sync.dma_start(x2c[:, 1], rsb3_out[1, :])
            x2cf = post.tile([HB, 2], fp32, tag="x2cf", name="x2cf")
            nc.vector.tensor_copy(x2cf[:], x2c[:])
            for half, rsa_out in ((0, rsa_outA), (1, rsa_outB)):
                gxy = post.tile([HB, N], bf16, tag=f"gxy{half}", name=f"gxy{half}")
                nc.sync.dma_start(gxy[:], rsa_out[:])
                exy = post.tile([HB, N], bf16, tag=f"exy{half}", name=f"exy{half}")
                nc.scalar.activation(
                    exy[:], gxy[:], mybir.ActivationFunctionType.Exp,
                    bias=x2cf[:, half:half + 1],
                    accum_out=accs[0:HB, 20 + half:21 + half],
                )

            # ---- discriminator tail (my 320 rows) ----
            h1c = post.tile([128, 320], bf16, tag="h1c", name="h1c")
            nc.sync.dma_start(h1c[:], rsb1_out[:])
            h1r = post.tile([128, 320], fp32, tag="h1r", name="h1r")
            nc.scalar.activation(
                h1r[:], h1c[:], mybir.ActivationFunctionType.Relu, bias=db1s,
            )
            l2ps = psmall.tile([64, 320], fp32, tag="l2ps", name="l2ps")
            nc.tensor.matmul(l2ps[:], dw2s, h1r[:], start=True, stop=True)
            h2r = post.tile([64, 320], fp32, tag="h2r", name="h2r")
            nc.scalar.activation(
                h2r[:], l2ps[:], mybir.ActivationFunctionType.Relu, bias=db2s,
            )
            l3ps = psmall.tile([2, 320], fp32, tag="l3ps", name="l3ps")
            nc.tensor.matmul(l3ps[:], dw3s, h2r[:], start=True, stop=True)
            sg = post.tile([2, 320], fp32, tag="sgm", name="sgm")
            nc.scalar.activation(
                sg[:], l3ps[:], mybir.ActivationFunctionType.Sigmoid, bias=db3s,
            )
            # softplus(x) = ln(1 + e^x); x in (0,1) here so no overflow concerns
            spe = post.tile([2, 320], fp32, tag="spe", name="spe")
            nc.scalar.activation(
                spe[:], sg[:], mybir.ActivationFunctionType.Exp,
            )
            sp = post.tile([2, 320], fp32, tag="sp", name="sp")
            nc.scalar.activation(
                sp[:], spe[:], mybir.ActivationFunctionType.Ln, bias=1.0,
                accum_out=accs[0:2, 22:23],
            )

            # ---- classifier tail (my 160 rows) ----
            c1c = post.tile([128, 160], fp32, tag="c1c", name="c1c")
            nc.sync.dma_start(c1c[:], rsb2_out[:])
            c1a = post.tile([128, 160], fp32, tag="c1a", name="c1a")
            nc.scalar.add(c1a[:], c1c[:], cb1s)
            cl2ps = psmall.tile([32, 160], fp32, tag="cl2ps", name="cl2ps")
            nc.tensor.matmul(cl2ps[:], cw2s, c1a[:], start=True, stop=True)
            c2a = post.tile([32, 160], fp32, tag="c2a", name="c2a")
            nc.scalar.add(c2a[:], cl2ps[:], cb2s)
            cl3ps = psmall.tile([3, 160], fp32, tag="cl3ps", name="cl3ps")
            nc.tensor.matmul(cl3ps[:], cw3s, c2a[:], start=True, stop=True)
            predsT = post.tile([3, 160], fp32, tag="predsTt", name="predsTt")
            nc.scalar.add(predsT[:], cl3ps[:], cb3s)

            nc.sync.dma_start(predsT_o[:], predsT[:])
            nc.sync.dma_start(accs_o[:], accs[:])
            ctx_ps4.__exit__(None, None, None)
            ctx_post.__exit__(None, None, None)

    nc.compile()
    return nc


def _get_program():
    if "nc" not in _CACHE:
        _CACHE["nc"] = _build_program()
    return _CACHE["nc"]


def _prep_inputs(source_features, target_features,
                 dW1, db1, dW2, db2, dW3, db3,
                 cW1, cb1, cW2, cb2, cW3, cb3):
    """Host-side shard: transpose + bf16-cast feature slices per core."""
    src_b = np.ascontiguousarray(source_features, dtype=np.float32).astype(BF16)
    tgt_b = np.ascontiguousarray(target_features, dtype=np.float32).astype(BF16)
    w1_b = np.asarray(dW1, dtype=np.float32).astype(BF16)
    cw1_b = np.asarray(cW1, dtype=np.float32).astype(BF16)

    pars = np.zeros((128, 107), np.float32)
    pars[:, 0] = np.asarray(db1, np.float32)
    pars[:, 1:65] = np.asarray(dW2, np.float32)
    pars[0:64, 65] = np.asarray(db2, np.float32)
    pars[0:64, 66:68] = np.asarray(dW3, np.float32)
    pars[0:2, 68] = np.asarray(db3, np.float32)
    pars[:, 69] = np.asarray(cb1, np.float32)
    pars[:, 70:102] = np.asarray(cW2, np.float32)
    pars[0:32, 102] = np.asarray(cb2, np.float32)
    pars[0:32, 103:106] = np.asarray(cW3, np.float32)
    pars[0:3, 106] = np.asarray(cb3, np.float32)
    smalls = {"params": pars}
    in_maps = []
    for c in range(NCORES):
        sl = slice(FS * c, FS * (c + 1))
        in_maps.append({
            "xt": np.ascontiguousarray(src_b[:, sl].T),
            "yt": np.ascontiguousarray(tgt_b[:, sl].T),
            "w1": np.ascontiguousarray(w1_b[sl, :]),
            "cw1": np.ascontiguousarray(cw1_b[sl, :]),
            **smalls,
        })
    return in_maps


def kernel(source_features, target_features, label,
           dW1, db1, dW2, db2, dW3, db3,
           cW1, cb1, cW2, cb2, cW3, cb3,
           _want_results=False):
    from concourse.bass_utils import run_bass_kernel_spmd

    nc = _get_program()
    in_maps = _prep_inputs(source_features, target_features,
                           dW1, db1, dW2, db2, dW3, db3,
                           cW1, cb1, cW2, cb2, cW3, cb3)
    res = run_bass_kernel_spmd(nc, in_maps, core_ids=list(range(NCORES)))
    results = res.results

    # ---------------- host epilogue ----------------
    preds = np.concatenate(
        [results[c]["predsT"].T for c in range(NCORES)], axis=0
    ).astype(np.float32)  # [1280, 3]

    acc0 = results[0]["accs"].astype(np.float64)
    kx_sum = acc0[:, 0:R].sum()
    ky_sum = acc0[:, R:2 * R].sum()
    kxy_sum = 0.0
    disc_total = 0.0
    for c in range(NCORES):
        a = results[c]["accs"].astype(np.float64)
        kxy_sum += a[:, 20].sum() + a[:, 21].sum()
        disc_total += a[0, 22] + a[1, 22]

    nl = ml = NLOC
    lmmd = (R / (nl * (nl - 1)) * kx_sum
            - 2.0 / (nl * ml) * kxy_sum
            + R / (ml * (ml - 1)) * ky_sum)
    mmd_loss = np.float32(lmmd / (R * R))

    discriminator_loss = np.float32(disc_total / (2 * N * 2))

    lab = np.asarray(label).astype(np.int64).reshape(-1)
    p64 = preds.astype(np.float64)
    pmax = p64.max(axis=1, keepdims=True)
    logp = p64 - pmax - np.log(np.exp(p64 - pmax).sum(axis=1, keepdims=True))
    class_loss = -logp[np.arange(N), lab].mean()
    l2 = sum(
        (np.asarray(t, np.float64) ** 2).sum()
        for t in (cW1, cb1, cW2, cb2, cW3, cb3)
    )
    classifier_loss = np.float32(class_loss + L2_LAMBDA * l2)
    loss = np.float32(np.float32(discriminator_loss) + np.float32(classifier_loss))

    out = (preds, classifier_loss, discriminator_loss, loss, mmd_loss)
    if _want_results:
        return out, res
    return out


if __name__ == "__main__":
    rng = np.random.default_rng(0)
    inp = {
        "source_features": rng.standard_normal((N, F), dtype=np.float32),
        "target_features": rng.standard_normal((N, F), dtype=np.float32),
        "label": rng.integers(0, 3, N).astype(np.int32),
        "dW1": 0.02 * rng.standard_normal((F, 128), dtype=np.float32),
        "db1": 0.02 * rng.standard_normal(128).astype(np.float32),
        "dW2": 0.02 * rng.standard_normal((128, 64), dtype=np.float32),
        "db2": 0.02 * rng.standard_normal(64).astype(np.float32),
        "dW3": 0.02 * rng.standard_normal((64, 2), dtype=np.float32),
        "db3": 0.02 * rng.standard_normal(2).astype(np.float32),
        "cW1": 0.02 * rng.standard_normal((F, 128), dtype=np.float32),
        "cb1": 0.02 * rng.standard_normal(128).astype(np.float32),
        "cW2": 0.02 * rng.standard_normal((128, 32), dtype=np.float32),
        "cb2": 0.02 * rng.standard_normal(32).astype(np.float32),
        "cW3": 0.02 * rng.standard_normal((32, 3), dtype=np.float32),
        "cb3": 0.02 * rng.standard_normal(3).astype(np.float32),
    }
    outs = kernel(**inp)
    for o in outs:
        print(np.asarray(o).shape, np.asarray(o).reshape(-1)[:4])


# revision 19
# speedup vs baseline: 1.1241x; 1.1241x over previous
"""Trainium2 Bass kernel for nn_Domain_adaptation (LMMD + discriminator/classifier losses).

Strategy (8 NeuronCores, feature-parallel):
  - The feature dim F=32768 is sharded 8x (4096 per core). Each core holds the
    transposed bf16 slices xT/yT = [4096, 1280] of source/target features.
  - Each core computes partial Gram matrices (src x tgt cross-Gram for Kxy,
    10 per-region self-Grams each for Kx / Ky), partial first-layer MLP outputs
    (dom @ dW1, tgt @ cW1), and partial squared-norm rows, all over its feature
    slice, on the PE array in bf16 (products of bf16 are exact in f32 PSUM).
  - A rank-1 term  -0.5 * ||col||^2  is folded into every Gram accumulation, so
    post-collective tiles hold  arg_pq = dot(p,q) - 0.5*||q||^2  directly.
  - Collectives (on-chip, overlap with compute):
      RS-A : ReduceScatter cross-Gram   [1280,1280] bf16 -> [160,1280]/core
      RS-B1: ReduceScatter h1T blocks   [1024, 320] f32  -> [128,320]/core
      RS-B2: ReduceScatter c1T blocks   [1024, 160] f32  -> [128,160]/core
      RS-B3: ReduceScatter x2 row       [1280, 1]  f32  -> [160,1]/core
      AR-C : AllReduce Kx/Ky self-Grams [2560, 128] bf16 (all cores get all)
  - exp(-d/2s^2): ACT engine, arg = G'_pq + bias_p with bias_p = -0.5*||p||^2.
    For Kx/Ky the bias is re-extracted from the *summed* Gram's own diagonal, so
    the diagonal argument is exactly 0 -> exp = 1 exactly (off-diagonals are
    ~-30000 and underflow to 0, as in the reference).
  - Each core finishes its batch slice of the discriminator/classifier MLPs
    (tiny f32 matmuls) and emits preds rows plus per-partition partial sums.
  - Host: gathers preds, sums partial scalars, computes log-softmax NLL, the L2
    term, and assembles the five outputs.
"""

import numpy as np
import ml_dtypes

BF16 = ml_dtypes.bfloat16

N = 1280          # batch (source and target)
F = 32768         # feature dim
NCORES = 8
FS = F // NCORES  # 4096 features per core
KCH = FS // 128   # 32 contraction chunks of 128
R = 10            # regions
NLOC = N // R     # 128 rows per region
L2_LAMBDA = 0.01
# 1/(2*sigma^2) with sigma=1 -> exp(-(d)*0.5); we compute arg = -0.5*d directly.

_CACHE = {}


def _build_program():
    import concourse.bass as bass
    import concourse.mybir as mybir
    import concourse.tile as tile
    from concourse import bacc
    from concourse.masks import make_identity

    fp32 = mybir.dt.float32
    bf16 = mybir.dt.bfloat16

    nc = bacc.Bacc(None, target_bir_lowering=False)

    # ---------------- I/O ----------------
    xt = nc.dram_tensor("xt", [FS, N], bf16, kind="ExternalInput")
    yt = nc.dram_tensor("yt", [FS, N], bf16, kind="ExternalInput")
    w1 = nc.dram_tensor("w1", [FS, 128], bf16, kind="ExternalInput")    # dW1 shard
    cw1 = nc.dram_tensor("cw1", [FS, 128], bf16, kind="ExternalInput")  # cW1 shard
    # packed small params [128, 107] f32: see _prep_inputs for column map
    params = nc.dram_tensor("params", [128, 107], fp32, kind="ExternalInput")

    predsT_o = nc.dram_tensor("predsT", [3, N // NCORES], fp32, kind="ExternalOutput")
    accs_o = nc.dram_tensor("accs", [128, 24], fp32, kind="ExternalOutput")

    rg = [list(range(NCORES))]
    HB = N // NCORES // 2  # 80: half-chunk rows of the split cross-Gram RS

    with tile.TileContext(nc) as tc:
        with (
            tc.tile_pool(name="small", bufs=1) as small,
            tc.tile_pool(name="dram", bufs=1, space="DRAM") as dram,
        ):
            # ------------- internal DRAM (collective bounce buffers) -------------
            rsa_inA = dram.tile([N // 2, N], bf16, tag="rsa_inA", name="rsa_inA")
            rsa_inB = dram.tile([N // 2, N], bf16, tag="rsa_inB", name="rsa_inB")
            rsa_outA = dram.tile([HB, N], bf16, tag="rsa_outA", name="rsa_outA")
            rsa_outB = dram.tile([HB, N], bf16, tag="rsa_outB", name="rsa_outB")
            rsb1_in = dram.tile([NCORES * 128, 320], bf16, tag="rsb1_in", name="rsb1_in")
            rsb1_out = dram.tile([128, 320], bf16, tag="rsb1_out", name="rsb1_out")
            rsb2_in = dram.tile([NCORES * 128, 160], fp32, tag="rsb2_in", name="rsb2_in")
            rsb2_out = dram.tile([128, 160], fp32, tag="rsb2_out", name="rsb2_out")
            rsb3_in = dram.tile([NCORES, 2, N // 2 // NCORES], bf16, tag="rsb3_in", name="rsb3_in")
            rsb3_out = dram.tile([2, N // 2 // NCORES], bf16, tag="rsb3_out", name="rsb3_out")
            arc_in = dram.tile([2 * R * 128, 128], bf16, tag="arc_in", name="arc_in")
            arc_out = dram.tile([2 * R * 128, 128], bf16, addr_space="Shared", tag="arc_out", name="arc_out")

            # ------------- persistent small tiles -------------
            pars = small.tile([128, 107], fp32, tag="pars", name="pars")
            nc.sync.dma_start(pars[:], params[:])
            # column map (host keeps in sync): db1 0; dw2 1:65; db2 65; dw3 66:68;
            # db3 68; cb1 69; cw2 70:102; cb2 102; cw3 103:106; cb3 106
            db1s = pars[:, 0:1]
            dw2s = pars[:, 1:65]
            db2s = pars[0:64, 65:66]
            dw3s = pars[0:64, 66:68]
            db3s = pars[0:2, 68:69]
            cb1s = pars[:, 69:70]
            cw2s = pars[:, 70:102]
            cb2s = pars[0:32, 102:103]
            cw3s = pars[0:32, 103:106]
            cb3s = pars[0:3, 106:107]

            ident = small.tile([128, 128], fp32, tag="ident", name="ident")
            make_identity(nc, ident[:])
            ones_col = small.tile([128, 1], fp32, tag="ones_col", name="ones_col")
            nc.gpsimd.memset(ones_col[:], 1.0)
            ones_row_b = small.tile([1, 128], bf16, tag="ones_row_b", name="ones_row_b")
            nc.gpsimd.memset(ones_row_b[:], 1.0)
            # -0.5*||col||^2 partial rows (bf16): [0:N]=src, [N:2N]=tgt
            xy2rowb = small.tile([1, 2 * N], bf16, tag="xy2rowb", name="xy2rowb")
            accs = small.tile([128, 24], fp32, tag="accs", name="accs")
            nc.gpsimd.memset(accs[:], 0.0)

            # ================= phase 1: load + MLP layer-1 partials =================
            ctx_main = tc.tile_pool(name="data", bufs=1)
            data = ctx_main.__enter__()
            ctx_p1 = tc.tile_pool(name="p1", bufs=1)
            p1 = ctx_p1.__enter__()

            # weights first (small), each as ONE dma: src rearranged so flat
            # iteration order matches the SBUF [p, k*128+j] layout
            w1all = p1.tile([128, KCH * 128], bf16, tag="w1all", name="w1all")
            c1all = p1.tile([128, KCH * 128], bf16, tag="c1all", name="c1all")
            nc.sync.dma_start(w1all[:], w1.rearrange("(k p) j -> p k j", p=128))
            nc.sync.dma_start(c1all[:], cw1.rearrange("(k p) j -> p k j", p=128))

            # dom chunks in groups of 4 (one dma per group per side): big enough
            # to amortize per-dma ring overhead, small enough to pipeline
            GRP = 4
            bigdom = data.tile([128, KCH, 2 * N], bf16, tag="bigdom", name="bigdom")
            for g in range(KCH // GRP):
                nc.sync.dma_start(
                    bigdom[:, GRP * g:GRP * (g + 1), 0:N],
                    xt[128 * GRP * g:128 * GRP * (g + 1), :].rearrange("(k p) j -> p k j", p=128),
                )
                nc.sync.dma_start(
                    bigdom[:, GRP * g:GRP * (g + 1), N:2 * N],
                    yt[128 * GRP * g:128 * GRP * (g + 1), :].rearrange("(k p) j -> p k j", p=128),
                )
            doms = [bigdom[:, k, :] for k in range(KCH)]

            ctx_ps1 = tc.tile_pool(name="ps1", bufs=1, space="PSUM")
            ps1 = ctx_ps1.__enter__()
            h1ps = [ps1.tile([128, 512], fp32, tag=f"h1ps{b}", name=f"h1ps{b}") for b in range(5)]
            c1ps = [ps1.tile([128, 512], fp32, tag=f"c1ps{b}", name=f"c1ps{b}") for b in range(3)]
            for k in range(KCH):
                for b in range(5):
                    nc.tensor.matmul(
                        h1ps[b][:], w1all[:, 128 * k:128 * (k + 1)],
                        doms[k][:, 512 * b:512 * (b + 1)],
                        start=(k == 0), stop=(k == KCH - 1),
                    )
                for b in range(3):
                    w = 512 if b < 2 else 256
                    nc.tensor.matmul(
                        c1ps[b][:, 0:w], c1all[:, 128 * k:128 * (k + 1)],
                        doms[k][:, N + 512 * b:N + 512 * b + w],
                        start=(k == 0), stop=(k == KCH - 1),
                    )

            # raw partial spill staging (bias/relu happen post-collective)
            h1sb = p1.tile([128, 8, 320], bf16, tag="h1sb", name="h1sb")
            c1sb = p1.tile([128, 8, 160], fp32, tag="c1sb", name="c1sb")
            for c in range(8):
                lo, hi = 320 * c, 320 * (c + 1)
                b0, b1 = lo // 512, (hi - 1) // 512
                for b in range(b0, b1 + 1):
                    s, e = max(lo, 512 * b), min(hi, 512 * (b + 1))
                    nc.scalar.copy(h1sb[:, c, s - lo:e - lo], h1ps[b][:, s - 512 * b:e - 512 * b])
                lo, hi = 160 * c, 160 * (c + 1)
                b0, b1 = lo // 512, (hi - 1) // 512
                for b in range(b0, b1 + 1):
                    s, e = max(lo, 512 * b), min(hi, 512 * (b + 1))
                    nc.scalar.copy(c1sb[:, c, s - lo:e - lo], c1ps[b][:, s - 512 * b:e - 512 * b])

            nc.sync.dma_start(rsb1_in.rearrange("(c p) j -> p c j", c=8), h1sb[:])
            nc.sync.dma_start(rsb2_in.rearrange("(c p) j -> p c j", c=8), c1sb[:])
            nc.gpsimd.collective_compute(
                "ReduceScatter", mybir.AluOpType.add, replica_groups=rg,
                ins=[rsb1_in[:]], outs=[rsb1_out[:]],
            )
            nc.gpsimd.collective_compute(
                "ReduceScatter", mybir.AluOpType.add, replica_groups=rg,
                ins=[rsb2_in[:]], outs=[rsb2_out[:]],
            )
            ctx_ps1.__exit__(None, None, None)
            ctx_p1.__exit__(None, None, None)

            # ================= phase 2a: per-region self-Grams (Kx, Ky) =================
            ctx_p2 = tc.tile_pool(name="p2", bufs=1)
            p2 = ctx_p2.__enter__()
            ctx_ps2 = tc.tile_pool(name="ps2", bufs=1, space="PSUM")
            ps2 = ctx_ps2.__enter__()
            sgps = [ps2.tile([128, 512], fp32, tag=f"sg{b}", name=f"sg{b}") for b in range(5)]

            def sg_ap(g):
                return sgps[g // 4][:, 128 * (g % 4):128 * (g % 4 + 1)]

            for k in range(KCH):
                for g in range(2 * R):
                    col = doms[k][:, 128 * g:128 * (g + 1)]
                    nc.tensor.matmul(sg_ap(g), col, col, start=(k == 0), stop=False)

            # -0.5 * diag rows (squared norms) from the partial Grams
            for g in range(2 * R):
                masked = p2.tile([128, 128], fp32, tag="masked", name="masked", bufs=2)
                nc.vector.scalar_tensor_tensor(
                    masked[:], sg_ap(g), 1.0, ident[:],
                    mybir.AluOpType.mult, mybir.AluOpType.mult,
                )
                rowp = ps2.tile([1, 128], fp32, tag="rowp", name="rowp", bufs=2)
                nc.tensor.matmul(rowp[:], ones_col[:], masked[:], start=True, stop=True)
                nc.scalar.activation(
                    xy2rowb[0:1, 128 * g:128 * (g + 1)], rowp[:],
                    mybir.ActivationFunctionType.Copy, scale=-0.5,
                )

            # fold rank-1 (-0.5*||q||^2) into each self-Gram, then stage as bf16
            gsall = p2.tile([128, 2 * R, 128], bf16, tag="gsall", name="gsall")
            for g in range(2 * R):
                nc.tensor.matmul(
                    sg_ap(g), ones_row_b[:], xy2rowb[0:1, 128 * g:128 * (g + 1)],
                    start=False, stop=True,
                )
                nc.scalar.copy(gsall[:, g, :], sg_ap(g))

            nc.scalar.dma_start(arc_in.rearrange("(g p) j -> p g j", g=2 * R), gsall[:])
            # x2 halves, permuted so RS chunk c = [rows 80c:80c+80 | rows 640+80c:...]
            nc.scalar.dma_start(rsb3_in[:, 0, :], xy2rowb[0:1, 0:N // 2])
            nc.scalar.dma_start(rsb3_in[:, 1, :], xy2rowb[0:1, N // 2:N])
            nc.gpsimd.collective_compute(
                "AllReduce", mybir.AluOpType.add, replica_groups=rg,
                ins=[arc_in[:]], outs=[arc_out[:]],
            )
            nc.gpsimd.collective_compute(
                "ReduceScatter", mybir.AluOpType.add, replica_groups=rg,
                ins=[rsb3_in[:]], outs=[rsb3_out[:]],
            )
            ctx_ps2.__exit__(None, None, None)

            # ================= phase 2b: cross-Gram Kxy (two halves) =================
            ctx_ps3 = tc.tile_pool(name="ps3", bufs=2, space="PSUM")
            ps3 = ctx_ps3.__enter__()
            widths = (512, 512, 256)
            for half, rsa_in in ((0, rsa_inA), (1, rsa_inB)):
                gmball = p2.tile([128, 5, N], bf16, tag=f"gmball{half}", name=f"gmball{half}")
                for mi in range(5):
                    m = 5 * half + mi
                    kxy = [
                        ps3.tile([128, 512], fp32, tag="kxy0", name="kxy0"),
                        ps3.tile([128, 512], fp32, tag="kxy1", name="kxy1"),
                        ps3.tile([128, 512], fp32, tag="kxy2", name="kxy2"),
                    ]
                    for k in range(KCH):
                        lhs = doms[k][:, 128 * m:128 * (m + 1)]
                        for b in range(3):
                            w = widths[b]
                            nc.tensor.matmul(
                                kxy[b][:, 0:w], lhs,
                                doms[k][:, N + 512 * b:N + 512 * b + w],
                                start=(k == 0), stop=False,
                            )
                    for b in range(3):
                        w = widths[b]
                        nc.tensor.matmul(
                            kxy[b][:, 0:w], ones_row_b[:],
                            xy2rowb[0:1, N + 512 * b:N + 512 * b + w],
                            start=False, stop=True,
                        )
                        nc.scalar.copy(gmball[:, mi, 512 * b:512 * b + w], kxy[b][:, 0:w])
                nc.scalar.dma_start(rsa_in.rearrange("(m p) j -> p m j", m=5), gmball[:])
                nc.gpsimd.collective_compute(
                    "ReduceScatter", mybir.AluOpType.add, replica_groups=rg,
                    ins=[rsa_in[:]], outs=[rsa_outA[:] if half == 0 else rsa_outB[:]],
                )
            ctx_ps3.__exit__(None, None, None)
            ctx_p2.__exit__(None, None, None)
            ctx_main.__exit__(None, None, None)

            # ================= post phase =================
            ctx_post = tc.tile_pool(name="post", bufs=1)
            post = ctx_post.__enter__()
            ctx_ps4 = tc.tile_pool(name="ps4", bufs=1, space="PSUM")
            psmall = ctx_ps4.__enter__()

            # ---- Kx/Ky exp-sums (every core, all regions, identical result) ----
            ggall = post.tile([128, 2 * R, 128], bf16, tag="ggall", name="ggall")
            nc.sync.dma_start(ggall[:], arc_out.rearrange("(g p) j -> p g j", g=2 * R))
            for g in range(2 * R):
                masked2 = post.tile([128, 128], fp32, tag="masked2", name="masked2", bufs=2)
                nc.vector.scalar_tensor_tensor(
                    masked2[:], ggall[:, g, :], 1.0, ident[:],
                    mybir.AluOpType.mult, mybir.AluOpType.mult,
                )
                negd = post.tile([128, 1], fp32, tag="negd", name="negd", bufs=2)
                nc.vector.tensor_reduce(
                    negd[:], masked2[:], mybir.AxisListType.X,
                    mybir.AluOpType.add, negate=True,
                )
                ex = post.tile([128, 128], bf16, tag="ex", name="ex", bufs=2)
                nc.scalar.activation(
                    ex[:], ggall[:, g, :], mybir.ActivationFunctionType.Exp,
                    bias=negd[:], accum_out=accs[:, g:g + 1],
                )

            # ---- Kxy chunk exp-sums (my two 80-row half-chunks) ----
            x2c = post.tile([HB, 2], bf16, tag="x2c", name="x2c")
            nc.sync.dma_start(x2c[:, 0], rsb3_out[0, :])
            nc.sync.dma_start(x2c[:, 1], rsb3_out[1, :])
            x2cf = post.tile([HB, 2], fp32, tag="x2cf", name="x2cf")
            nc.vector.tensor_copy(x2cf[:], x2c[:])
            for half, rsa_out in ((0, rsa_outA), (1, rsa_outB)):
                gxy = post.tile([HB, N], bf16, tag=f"gxy{half}", name=f"gxy{half}")
                nc.sync.dma_start(gxy[:], rsa_out[:])
                exy = post.tile([HB, N], bf16, tag=f"exy{half}", name=f"exy{half}")
                nc.scalar.activation(
                    exy[:], gxy[:], mybir.ActivationFunctionType.Exp,
                    bias=x2cf[:, half:half + 1],
                    accum_out=accs[0:HB, 20 + half:21 + half],
                )

            # ---- discriminator tail (my 320 rows) ----
            h1c = post.tile([128, 320], bf16, tag="h1c", name="h1c")
            nc.sync.dma_start(h1c[:], rsb1_out[:])
            h1r = post.tile([128, 320], fp32, tag="h1r", name="h1r")
            nc.scalar.activation(
                h1r[:], h1c[:], mybir.ActivationFunctionType.Relu, bias=db1s,
            )
            l2ps = psmall.tile([64, 320], fp32, tag="l2ps", name="l2ps")
            nc.tensor.matmul(l2ps[:], dw2s, h1r[:], start=True, stop=True)
            h2r = post.tile([64, 320], fp32, tag="h2r", name="h2r")
            nc.scalar.activation(
                h2r[:], l2ps[:], mybir.ActivationFunctionType.Relu, bias=db2s,
            )
            l3ps = psmall.tile([2, 320], fp32, tag="l3ps", name="l3ps")
            nc.tensor.matmul(l3ps[:], dw3s, h2r[:], start=True, stop=True)
            sg = post.tile([2, 320], fp32, tag="sgm", name="sgm")
            nc.scalar.activation(
                sg[:], l3ps[:], mybir.ActivationFunctionType.Sigmoid, bias=db3s,
            )
            # softplus(x) = ln(1 + e^x); x in (0,1) here so no overflow concerns
            spe = post.tile([2, 320], fp32, tag="spe", name="spe")
            nc.scalar.activation(
                spe[:], sg[:], mybir.ActivationFunctionType.Exp,
            )
            sp = post.tile([2, 320], fp32, tag="sp", name="sp")
            nc.scalar.activation(
                sp[:], spe[:], mybir.ActivationFunctionType.Ln, bias=1.0,
                accum_out=accs[0:2, 22:23],
            )

            # ---- classifier tail (my 160 rows) ----
            c1c = post.tile([128, 160], fp32, tag="c1c", name="c1c")
            nc.sync.dma_start(c1c[:], rsb2_out[:])
            c1a = post.tile([128, 160], fp32, tag="c1a", name="c1a")
            nc.scalar.add(c1a[:], c1c[:], cb1s)
            cl2ps = psmall.tile([32, 160], fp32, tag="cl2ps", name="cl2ps")
            nc.tensor.matmul(cl2ps[:], cw2s, c1a[:], start=True, stop=True)
            c2a = post.tile([32, 160], fp32, tag="c2a", name="c2a")
            nc.scalar.add(c2a[:], cl2ps[:], cb2s)
            cl3ps = psmall.tile([3, 160], fp32, tag="cl3ps", name="cl3ps")
            nc.tensor.matmul(cl3ps[:], cw3s, c2a[:], start=True, stop=True)
            predsT = post.tile([3, 160], fp32, tag="predsTt", name="predsTt")
            nc.scalar.add(predsT[:], cl3ps[:], cb3s)

            nc.sync.dma_start(predsT_o[:], predsT[:])
            nc.sync.dma_start(accs_o[:], accs[:])
            ctx_ps4.__exit__(None, None, None)
            ctx_post.__exit__(None, None, None)

    nc.compile()
    return nc


def _get_program():
    if "nc" not in _CACHE:
        _CACHE["nc"] = _build_program()
    return _CACHE["nc"]


def _prep_inputs(source_features, target_features,
                 dW1, db1, dW2, db2, dW3, db3,
                 cW1, cb1, cW2, cb2, cW3, cb3):
    """Host-side shard: transpose + bf16-cast feature slices per core."""
    src_b = np.ascontiguousarray(source_features, dtype=np.float32).astype(BF16)
    tgt_b = np.ascontiguousarray(target_features, dtype=np.float32).astype(BF16)
    w1_b = np.asarray(dW1, dtype=np.float32).astype(BF16)
    cw1_b = np.asarray(cW1, dtype=np.float32).astype(BF16)

    pars = np.zeros((128, 107), np.float32)
    pars[:, 0] = np.asarray(db1, np.float32)
    pars[:, 1:65] = np.asarray(dW2, np.float32)
    pars[0:64, 65] = np.asarray(db2, np.float32)
    pars[0:64, 66:68] = np.asarray(dW3, np.float32)
    pars[0:2, 68] = np.asarray(db3, np.float32)
    pars[:, 69] = np.asarray(cb1, np.float32)
    pars[:, 70:102] = np.asarray(cW2, np.float32)
    pars[0:32, 102] = np.asarray(cb2, np.float32)
    pars[0:32, 103:106] = np.asarray(cW3, np.float32)
    pars[0:3, 106] = np.asarray(cb3, np.float32)
    smalls = {"params": pars}
    in_maps = []
    for c in range(NCORES):
        sl = slice(FS * c, FS * (c + 1))
        in_maps.append({
            "xt": np.ascontiguousarray(src_b[:, sl].T),
            "yt": np.ascontiguousarray(tgt_b[:, sl].T),
            "w1": np.ascontiguousarray(w1_b[sl, :]),
            "cw1": np.ascontiguousarray(cw1_b[sl, :]),
            **smalls,
        })
    return in_maps


def kernel(source_features, target_features, label,
           dW1, db1, dW2, db2, dW3, db3,
           cW1, cb1, cW2, cb2, cW3, cb3,
           _want_results=False):
    from concourse.bass_utils import run_bass_kernel_spmd

    nc = _get_program()
    in_maps = _prep_inputs(source_features, target_features,
                           dW1, db1, dW2, db2, dW3, db3,
                           cW1, cb1, cW2, cb2, cW3, cb3)
    res = run_bass_kernel_spmd(nc, in_maps, core_ids=list(range(NCORES)))
    results = res.results

    # ---------------- host epilogue ----------------
    preds = np.concatenate(
        [results[c]["predsT"].T for c in range(NCORES)], axis=0
    ).astype(np.float32)  # [1280, 3]

    acc0 = results[0]["accs"].astype(np.float64)
    kx_sum = acc0[:, 0:R].sum()
    ky_sum = acc0[:, R:2 * R].sum()
    kxy_sum = 0.0
    disc_total = 0.0
    for c in range(NCORES):
        a = results[c]["accs"].astype(np.float64)
        kxy_sum += a[:, 20].sum() + a[:, 21].sum()
        disc_total += a[0, 22] + a[1, 22]

    nl = ml = NLOC
    lmmd = (R / (nl * (nl - 1)) * kx_sum
            - 2.0 / (nl * ml) * kxy_sum
            + R / (ml * (ml - 1)) * ky_sum)
    mmd_loss = np.float32(lmmd / (R * R))

    discriminator_loss = np.float32(disc_total / (2 * N * 2))

    lab = np.asarray(label).astype(np.int64).reshape(-1)
    p64 = preds.astype(np.float64)
    pmax = p64.max(axis=1, keepdims=True)
    logp = p64 - pmax - np.log(np.exp(p64 - pmax).sum(axis=1, keepdims=True))
    class_loss = -logp[np.arange(N), lab].mean()
    l2 = sum(
        (np.asarray(t, np.float64) ** 2).sum()
        for t in (cW1, cb1, cW2, cb2, cW3, cb3)
    )
    classifier_loss = np.float32(class_loss + L2_LAMBDA * l2)
    loss = np.float32(np.float32(discriminator_loss) + np.float32(classifier_loss))

    out = (preds, classifier_loss, discriminator_loss, loss, mmd_loss)
    if _want_results:
        return out, res
    return out


if __name__ == "__main__":
    rng = np.random.default_rng(0)
    inp = {
        "source_features": rng.standard_normal((N, F), dtype=np.float32),
        "target_features": rng.standard_normal((N, F), dtype=np.float32),
        "label": rng.integers(0, 3, N).astype(np.int32),
        "dW1": 0.02 * rng.standard_normal((F, 128), dtype=np.float32),
        "db1": 0.02 * rng.standard_normal(128).astype(np.float32),
        "dW2": 0.02 * rng.standard_normal((128, 64), dtype=np.float32),
        "db2": 0.02 * rng.standard_normal(64).astype(np.float32),
        "dW3": 0.02 * rng.standard_normal((64, 2), dtype=np.float32),
        "db3": 0.02 * rng.standard_normal(2).astype(np.float32),
        "cW1": 0.02 * rng.standard_normal((F, 128), dtype=np.float32),
        "cb1": 0.02 * rng.standard_normal(128).astype(np.float32),
        "cW2": 0.02 * rng.standard_normal((128, 32), dtype=np.float32),
        "cb2": 0.02 * rng.standard_normal(32).astype(np.float32),
        "cW3": 0.02 * rng.standard_normal((32, 3), dtype=np.float32),
        "cb3": 0.02 * rng.standard_normal(3).astype(np.float32),
    }
    outs = kernel(**inp)
    for o in outs:
        print(np.asarray(o).shape, np.asarray(o).reshape(-1)[:4])


# revision 21
# speedup vs baseline: 1.1374x; 1.0119x over previous
"""Trainium2 Bass kernel for nn_Domain_adaptation (LMMD + discriminator/classifier losses).

Strategy (8 NeuronCores, feature-parallel):
  - The feature dim F=32768 is sharded 8x (4096 per core). Each core holds the
    transposed bf16 slices xT/yT = [4096, 1280] of source/target features.
  - Each core computes partial Gram matrices (src x tgt cross-Gram for Kxy,
    10 per-region self-Grams each for Kx / Ky), partial first-layer MLP outputs
    (dom @ dW1, tgt @ cW1), and partial squared-norm rows, all over its feature
    slice, on the PE array in bf16 (products of bf16 are exact in f32 PSUM).
  - A rank-1 term  -0.5 * ||col||^2  is folded into every Gram accumulation, so
    post-collective tiles hold  arg_pq = dot(p,q) - 0.5*||q||^2  directly.
  - Collectives (on-chip, overlap with compute):
      RS-A : ReduceScatter cross-Gram   [1280,1280] bf16 -> [160,1280]/core
      RS-B1: ReduceScatter h1T blocks   [1024, 320] f32  -> [128,320]/core
      RS-B2: ReduceScatter c1T blocks   [1024, 160] f32  -> [128,160]/core
      RS-B3: ReduceScatter x2 row       [1280, 1]  f32  -> [160,1]/core
      AR-C : AllReduce Kx/Ky self-Grams [2560, 128] bf16 (all cores get all)
  - exp(-d/2s^2): ACT engine, arg = G'_pq + bias_p with bias_p = -0.5*||p||^2.
    For Kx/Ky the bias is re-extracted from the *summed* Gram's own diagonal, so
    the diagonal argument is exactly 0 -> exp = 1 exactly (off-diagonals are
    ~-30000 and underflow to 0, as in the reference).
  - Each core finishes its batch slice of the discriminator/classifier MLPs
    (tiny f32 matmuls) and emits preds rows plus per-partition partial sums.
  - Host: gathers preds, sums partial scalars, computes log-softmax NLL, the L2
    term, and assembles the five outputs.
"""

import numpy as np
import ml_dtypes

BF16 = ml_dtypes.bfloat16

N = 1280          # batch (source and target)
F = 32768         # feature dim
NCORES = 8
FS = F // NCORES  # 4096 features per core
KCH = FS // 128   # 32 contraction chunks of 128
R = 10            # regions
NLOC = N // R     # 128 rows per region
L2_LAMBDA = 0.01
# 1/(2*sigma^2) with sigma=1 -> exp(-(d)*0.5); we compute arg = -0.5*d directly.

_CACHE = {}


def _build_program():
    import concourse.bass as bass
    import concourse.mybir as mybir
    import concourse.tile as tile
    from concourse import bacc
    from concourse.masks import make_identity

    fp32 = mybir.dt.float32
    bf16 = mybir.dt.bfloat16

    nc = bacc.Bacc(None, target_bir_lowering=False)

    # ---------------- I/O ----------------
    xt = nc.dram_tensor("xt", [FS, N], bf16, kind="ExternalInput")
    yt = nc.dram_tensor("yt", [FS, N], bf16, kind="ExternalInput")
    w1 = nc.dram_tensor("w1", [FS, 128], bf16, kind="ExternalInput")    # dW1 shard
    cw1 = nc.dram_tensor("cw1", [FS, 128], bf16, kind="ExternalInput")  # cW1 shard
    # packed small params [128, 107] f32: see _prep_inputs for column map
    params = nc.dram_tensor("params", [128, 107], fp32, kind="ExternalInput")

    predsT_o = nc.dram_tensor("predsT", [3, N // NCORES], fp32, kind="ExternalOutput")
    accs_o = nc.dram_tensor("accs", [128, 24], fp32, kind="ExternalOutput")

    rg = [list(range(NCORES))]
    HB = N // NCORES // 2  # 80: half-chunk rows of the split cross-Gram RS

    with tile.TileContext(nc) as tc:
        with (
            tc.tile_pool(name="small", bufs=1) as small,
            tc.tile_pool(name="dram", bufs=1, space="DRAM") as dram,
        ):
            # ------------- internal DRAM (collective bounce buffers) -------------
            rsa_inA = dram.tile([896, N], bf16, tag="rsa_inA", name="rsa_inA")
            rsa_inB = dram.tile([384, N], bf16, tag="rsa_inB", name="rsa_inB")
            rsa_outA = dram.tile([112, N], bf16, tag="rsa_outA", name="rsa_outA")
            rsa_outB = dram.tile([48, N], bf16, tag="rsa_outB", name="rsa_outB")
            rsb1_in = dram.tile([NCORES * 128, 320], bf16, tag="rsb1_in", name="rsb1_in")
            rsb1_out = dram.tile([128, 320], bf16, tag="rsb1_out", name="rsb1_out")
            rsb2_in = dram.tile([NCORES * 128, 160], fp32, tag="rsb2_in", name="rsb2_in")
            rsb2_out = dram.tile([128, 160], fp32, tag="rsb2_out", name="rsb2_out")
            rsb3_in = dram.tile([NCORES, 160], bf16, tag="rsb3_in", name="rsb3_in")
            rsb3_out = dram.tile([1, 160], bf16, tag="rsb3_out", name="rsb3_out")
            arc_in = dram.tile([2 * R * 128, 128], bf16, tag="arc_in", name="arc_in")
            arc_out = dram.tile([2 * R * 128, 128], bf16, addr_space="Shared", tag="arc_out", name="arc_out")

            # ------------- persistent small tiles -------------
            pars = small.tile([128, 107], fp32, tag="pars", name="pars")
            nc.sync.dma_start(pars[:], params[:])
            # column map (host keeps in sync): db1 0; dw2 1:65; db2 65; dw3 66:68;
            # db3 68; cb1 69; cw2 70:102; cb2 102; cw3 103:106; cb3 106
            db1s = pars[:, 0:1]
            dw2s = pars[:, 1:65]
            db2s = pars[0:64, 65:66]
            dw3s = pars[0:64, 66:68]
            db3s = pars[0:2, 68:69]
            cb1s = pars[:, 69:70]
            cw2s = pars[:, 70:102]
            cb2s = pars[0:32, 102:103]
            cw3s = pars[0:32, 103:106]
            cb3s = pars[0:3, 106:107]

            ident = small.tile([128, 128], fp32, tag="ident", name="ident")
            make_identity(nc, ident[:])
            ones_col = small.tile([128, 1], fp32, tag="ones_col", name="ones_col")
            nc.gpsimd.memset(ones_col[:], 1.0)
            ones_row_b = small.tile([1, 128], bf16, tag="ones_row_b", name="ones_row_b")
            nc.gpsimd.memset(ones_row_b[:], 1.0)
            # -0.5*||col||^2 partial rows (bf16): [0:N]=src, [N:2N]=tgt
            xy2rowb = small.tile([1, 2 * N], bf16, tag="xy2rowb", name="xy2rowb")
            accs = small.tile([128, 24], fp32, tag="accs", name="accs")
            nc.gpsimd.memset(accs[:], 0.0)

            # ================= phase 1: load + MLP layer-1 partials =================
            ctx_main = tc.tile_pool(name="data", bufs=1)
            data = ctx_main.__enter__()
            ctx_p1 = tc.tile_pool(name="p1", bufs=1)
            p1 = ctx_p1.__enter__()

            # weights first (small), each as ONE dma: src rearranged so flat
            # iteration order matches the SBUF [p, k*128+j] layout
            w1all = p1.tile([128, KCH * 128], bf16, tag="w1all", name="w1all")
            c1all = p1.tile([128, KCH * 128], bf16, tag="c1all", name="c1all")
            nc.sync.dma_start(w1all[:], w1.rearrange("(k p) j -> p k j", p=128))
            nc.sync.dma_start(c1all[:], cw1.rearrange("(k p) j -> p k j", p=128))

            # dom chunks in groups of 4 (one dma per group per side): big enough
            # to amortize per-dma ring overhead, small enough to pipeline
            GRP = 4
            bigdom = data.tile([128, KCH, 2 * N], bf16, tag="bigdom", name="bigdom")
            for g in range(KCH // GRP):
                nc.sync.dma_start(
                    bigdom[:, GRP * g:GRP * (g + 1), 0:N],
                    xt[128 * GRP * g:128 * GRP * (g + 1), :].rearrange("(k p) j -> p k j", p=128),
                )
                nc.sync.dma_start(
                    bigdom[:, GRP * g:GRP * (g + 1), N:2 * N],
                    yt[128 * GRP * g:128 * GRP * (g + 1), :].rearrange("(k p) j -> p k j", p=128),
                )
            doms = [bigdom[:, k, :] for k in range(KCH)]

            ctx_ps1 = tc.tile_pool(name="ps1", bufs=1, space="PSUM")
            ps1 = ctx_ps1.__enter__()
            h1ps = [ps1.tile([128, 512], fp32, tag=f"h1ps{b}", name=f"h1ps{b}") for b in range(5)]
            c1ps = [ps1.tile([128, 512], fp32, tag=f"c1ps{b}", name=f"c1ps{b}") for b in range(3)]
            for k in range(KCH):
                for b in range(5):
                    nc.tensor.matmul(
                        h1ps[b][:], w1all[:, 128 * k:128 * (k + 1)],
                        doms[k][:, 512 * b:512 * (b + 1)],
                        start=(k == 0), stop=(k == KCH - 1),
                    )
                for b in range(3):
                    w = 512 if b < 2 else 256
                    nc.tensor.matmul(
                        c1ps[b][:, 0:w], c1all[:, 128 * k:128 * (k + 1)],
                        doms[k][:, N + 512 * b:N + 512 * b + w],
                        start=(k == 0), stop=(k == KCH - 1),
                    )

            # raw partial spill staging (bias/relu happen post-collective)
            h1sb = p1.tile([128, 8, 320], bf16, tag="h1sb", name="h1sb")
            c1sb = p1.tile([128, 8, 160], fp32, tag="c1sb", name="c1sb")
            for c in range(8):
                lo, hi = 320 * c, 320 * (c + 1)
                b0, b1 = lo // 512, (hi - 1) // 512
                for b in range(b0, b1 + 1):
                    s, e = max(lo, 512 * b), min(hi, 512 * (b + 1))
                    nc.scalar.copy(h1sb[:, c, s - lo:e - lo], h1ps[b][:, s - 512 * b:e - 512 * b])
                lo, hi = 160 * c, 160 * (c + 1)
                b0, b1 = lo // 512, (hi - 1) // 512
                for b in range(b0, b1 + 1):
                    s, e = max(lo, 512 * b), min(hi, 512 * (b + 1))
                    nc.scalar.copy(c1sb[:, c, s - lo:e - lo], c1ps[b][:, s - 512 * b:e - 512 * b])

            nc.sync.dma_start(rsb1_in.rearrange("(c p) j -> p c j", c=8), h1sb[:])
            nc.sync.dma_start(rsb2_in.rearrange("(c p) j -> p c j", c=8), c1sb[:])
            nc.gpsimd.collective_compute(
                "ReduceScatter", mybir.AluOpType.add, replica_groups=rg,
                ins=[rsb1_in[:]], outs=[rsb1_out[:]],
            )
            nc.gpsimd.collective_compute(
                "ReduceScatter", mybir.AluOpType.add, replica_groups=rg,
                ins=[rsb2_in[:]], outs=[rsb2_out[:]],
            )
            ctx_ps1.__exit__(None, None, None)
            ctx_p1.__exit__(None, None, None)

            # ================= phase 2a: per-region self-Grams (Kx, Ky) =================
            ctx_p2 = tc.tile_pool(name="p2", bufs=1)
            p2 = ctx_p2.__enter__()
            ctx_ps2 = tc.tile_pool(name="ps2", bufs=1, space="PSUM")
            ps2 = ctx_ps2.__enter__()
            sgps = [ps2.tile([128, 512], fp32, tag=f"sg{b}", name=f"sg{b}") for b in range(5)]

            def sg_ap(g):
                return sgps[g // 4][:, 128 * (g % 4):128 * (g % 4 + 1)]

            for k in range(KCH):
                for g in range(2 * R):
                    col = doms[k][:, 128 * g:128 * (g + 1)]
                    nc.tensor.matmul(sg_ap(g), col, col, start=(k == 0), stop=False)

            # -0.5 * diag rows (squared norms) from the partial Grams
            for g in range(2 * R):
                masked = p2.tile([128, 128], fp32, tag="masked", name="masked", bufs=2)
                nc.vector.scalar_tensor_tensor(
                    masked[:], sg_ap(g), 1.0, ident[:],
                    mybir.AluOpType.mult, mybir.AluOpType.mult,
                )
                rowp = ps2.tile([1, 128], fp32, tag="rowp", name="rowp", bufs=2)
                nc.tensor.matmul(rowp[:], ones_col[:], masked[:], start=True, stop=True)
                nc.scalar.activation(
                    xy2rowb[0:1, 128 * g:128 * (g + 1)], rowp[:],
                    mybir.ActivationFunctionType.Copy, scale=-0.5,
                )

            # fold rank-1 (-0.5*||q||^2) into each self-Gram, then stage as bf16
            gsall = p2.tile([128, 2 * R, 128], bf16, tag="gsall", name="gsall")
            for g in range(2 * R):
                nc.tensor.matmul(
                    sg_ap(g), ones_row_b[:], xy2rowb[0:1, 128 * g:128 * (g + 1)],
                    start=False, stop=True,
                )
                nc.scalar.copy(gsall[:, g, :], sg_ap(g))

            nc.scalar.dma_start(arc_in.rearrange("(g p) j -> p g j", g=2 * R), gsall[:])
            # x2 halves, permuted so RS chunk c = [rows 80c:80c+80 | rows 640+80c:...]
            nc.scalar.dma_start(rsb3_in[:, 0:112], xy2rowb[0:1, 0:896])
            nc.scalar.dma_start(rsb3_in[:, 112:160], xy2rowb[0:1, 896:N])
            nc.gpsimd.collective_compute(
                "AllReduce", mybir.AluOpType.add, replica_groups=rg,
                ins=[arc_in[:]], outs=[arc_out[:]],
            )
            nc.gpsimd.collective_compute(
                "ReduceScatter", mybir.AluOpType.add, replica_groups=rg,
                ins=[rsb3_in[:]], outs=[rsb3_out[:]],
            )
            ctx_ps2.__exit__(None, None, None)

            # ================= phase 2b: cross-Gram Kxy (two halves) =================
            ctx_ps3 = tc.tile_pool(name="ps3", bufs=2, space="PSUM")
            ps3 = ctx_ps3.__enter__()
            widths = (512, 512, 256)
            MSPLIT = ((0, 7), (7, 3))
            for half, rsa_in in ((0, rsa_inA), (1, rsa_inB)):
                m0, nm = MSPLIT[half]
                gmball = p2.tile([128, nm, N], bf16, tag=f"gmball{half}", name=f"gmball{half}")
                for mi in range(nm):
                    m = m0 + mi
                    kxy = [
                        ps3.tile([128, 512], fp32, tag="kxy0", name="kxy0"),
                        ps3.tile([128, 512], fp32, tag="kxy1", name="kxy1"),
                        ps3.tile([128, 512], fp32, tag="kxy2", name="kxy2"),
                    ]
                    for k in range(KCH):
                        lhs = doms[k][:, 128 * m:128 * (m + 1)]
                        for b in range(3):
                            w = widths[b]
                            nc.tensor.matmul(
                                kxy[b][:, 0:w], lhs,
                                doms[k][:, N + 512 * b:N + 512 * b + w],
                                start=(k == 0), stop=False,
                            )
                    for b in range(3):
                        w = widths[b]
                        nc.tensor.matmul(
                            kxy[b][:, 0:w], ones_row_b[:],
                            xy2rowb[0:1, N + 512 * b:N + 512 * b + w],
                            start=False, stop=True,
                        )
                        nc.scalar.copy(gmball[:, mi, 512 * b:512 * b + w], kxy[b][:, 0:w])
                nc.scalar.dma_start(rsa_in.rearrange("(m p) j -> p m j", m=nm), gmball[:])
                nc.gpsimd.collective_compute(
                    "ReduceScatter", mybir.AluOpType.add, replica_groups=rg,
                    ins=[rsa_in[:]], outs=[rsa_outA[:] if half == 0 else rsa_outB[:]],
                )
            ctx_ps3.__exit__(None, None, None)
            ctx_p2.__exit__(None, None, None)
            ctx_main.__exit__(None, None, None)

            # ================= post phase =================
            ctx_post = tc.tile_pool(name="post", bufs=1)
            post = ctx_post.__enter__()
            ctx_ps4 = tc.tile_pool(name="ps4", bufs=1, space="PSUM")
            psmall = ctx_ps4.__enter__()

            # ---- Kx/Ky exp-sums (every core, all regions, identical result) ----
            ggall = post.tile([128, 2 * R, 128], bf16, tag="ggall", name="ggall")
            nc.sync.dma_start(ggall[:], arc_out.rearrange("(g p) j -> p g j", g=2 * R))
            for g in range(2 * R):
                masked2 = post.tile([128, 128], fp32, tag="masked2", name="masked2", bufs=2)
                nc.vector.scalar_tensor_tensor(
                    masked2[:], ggall[:, g, :], 1.0, ident[:],
                    mybir.AluOpType.mult, mybir.AluOpType.mult,
                )
                negd = post.tile([128, 1], fp32, tag="negd", name="negd", bufs=2)
                nc.vector.tensor_reduce(
                    negd[:], masked2[:], mybir.AxisListType.X,
                    mybir.AluOpType.add, negate=True,
                )
                ex = post.tile([128, 128], bf16, tag="ex", name="ex", bufs=2)
                nc.scalar.activation(
                    ex[:], ggall[:, g, :], mybir.ActivationFunctionType.Exp,
                    bias=negd[:], accum_out=accs[:, g:g + 1],
                )

            # ---- Kxy chunk exp-sums (my two 80-row half-chunks) ----
            x2c = post.tile([112, 2], bf16, tag="x2c", name="x2c")
            nc.sync.dma_start(x2c[0:112, 0], rsb3_out[0, 0:112])
            nc.sync.dma_start(x2c[0:48, 1], rsb3_out[0, 112:160])
            x2cf = post.tile([112, 2], fp32, tag="x2cf", name="x2cf")
            nc.vector.tensor_copy(x2cf[:], x2c[:])
            for half, rsa_out, hh in ((0, rsa_outA, 112), (1, rsa_outB, 48)):
                gxy = post.tile([hh, N], bf16, tag=f"gxy{half}", name=f"gxy{half}")
                nc.sync.dma_start(gxy[:], rsa_out[:])
                exy = post.tile([hh, N], bf16, tag=f"exy{half}", name=f"exy{half}")
                nc.scalar.activation(
                    exy[:], gxy[:], mybir.ActivationFunctionType.Exp,
                    bias=x2cf[0:hh, half:half + 1],
                    accum_out=accs[0:hh, 20 + half:21 + half],
                )

            # ---- discriminator tail (my 320 rows) ----
            h1c = post.tile([128, 320], bf16, tag="h1c", name="h1c")
            nc.sync.dma_start(h1c[:], rsb1_out[:])
            h1r = post.tile([128, 320], fp32, tag="h1r", name="h1r")
            nc.scalar.activation(
                h1r[:], h1c[:], mybir.ActivationFunctionType.Relu, bias=db1s,
            )
            l2ps = psmall.tile([64, 320], fp32, tag="l2ps", name="l2ps")
            nc.tensor.matmul(l2ps[:], dw2s, h1r[:], start=True, stop=True)
            h2r = post.tile([64, 320], fp32, tag="h2r", name="h2r")
            nc.scalar.activation(
                h2r[:], l2ps[:], mybir.ActivationFunctionType.Relu, bias=db2s,
            )
            l3ps = psmall.tile([2, 320], fp32, tag="l3ps", name="l3ps")
            nc.tensor.matmul(l3ps[:], dw3s, h2r[:], start=True, stop=True)
            sg = post.tile([2, 320], fp32, tag="sgm", name="sgm")
            nc.scalar.activation(
                sg[:], l3ps[:], mybir.ActivationFunctionType.Sigmoid, bias=db3s,
            )
            # softplus(x) = ln(1 + e^x); x in (0,1) here so no overflow concerns
            spe = post.tile([2, 320], fp32, tag="spe", name="spe")
            nc.scalar.activation(
                spe[:], sg[:], mybir.ActivationFunctionType.Exp,
            )
            sp = post.tile([2, 320], fp32, tag="sp", name="sp")
            nc.scalar.activation(
                sp[:], spe[:], mybir.ActivationFunctionType.Ln, bias=1.0,
                accum_out=accs[0:2, 22:23],
            )

            # ---- classifier tail (my 160 rows) ----
            c1c = post.tile([128, 160], fp32, tag="c1c", name="c1c")
            nc.sync.dma_start(c1c[:], rsb2_out[:])
            c1a = post.tile([128, 160], fp32, tag="c1a", name="c1a")
            nc.scalar.add(c1a[:], c1c[:], cb1s)
            cl2ps = psmall.tile([32, 160], fp32, tag="cl2ps", name="cl2ps")
            nc.tensor.matmul(cl2ps[:], cw2s, c1a[:], start=True, stop=True)
            c2a = post.tile([32, 160], fp32, tag="c2a", name="c2a")
            nc.scalar.add(c2a[:], cl2ps[:], cb2s)
            cl3ps = psmall.tile([3, 160], fp32, tag="cl3ps", name="cl3ps")
            nc.tensor.matmul(cl3ps[:], cw3s, c2a[:], start=True, stop=True)
            predsT = post.tile([3, 160], fp32, tag="predsTt", name="predsTt")
            nc.scalar.add(predsT[:], cl3ps[:], cb3s)

            nc.sync.dma_start(predsT_o[:], predsT[:])
            nc.sync.dma_start(accs_o[:], accs[:])
            ctx_ps4.__exit__(None, None, None)
            ctx_post.__exit__(None, None, None)

    nc.compile()
    return nc


def _get_program():
    if "nc" not in _CACHE:
        _CACHE["nc"] = _build_program()
    return _CACHE["nc"]


def _prep_inputs(source_features, target_features,
                 dW1, db1, dW2, db2, dW3, db3,
                 cW1, cb1, cW2, cb2, cW3, cb3):
    """Host-side shard: transpose + bf16-cast feature slices per core."""
    src_b = np.ascontiguousarray(source_features, dtype=np.float32).astype(BF16)
    tgt_b = np.ascontiguousarray(target_features, dtype=np.float32).astype(BF16)
    w1_b = np.asarray(dW1, dtype=np.float32).astype(BF16)
    cw1_b = np.asarray(cW1, dtype=np.float32).astype(BF16)

    pars = np.zeros((128, 107), np.float32)
    pars[:, 0] = np.asarray(db1, np.float32)
    pars[:, 1:65] = np.asarray(dW2, np.float32)
    pars[0:64, 65] = np.asarray(db2, np.float32)
    pars[0:64, 66:68] = np.asarray(dW3, np.float32)
    pars[0:2, 68] = np.asarray(db3, np.float32)
    pars[:, 69] = np.asarray(cb1, np.float32)
    pars[:, 70:102] = np.asarray(cW2, np.float32)
    pars[0:32, 102] = np.asarray(cb2, np.float32)
    pars[0:32, 103:106] = np.asarray(cW3, np.float32)
    pars[0:3, 106] = np.asarray(cb3, np.float32)
    smalls = {"params": pars}
    in_maps = []
    for c in range(NCORES):
        sl = slice(FS * c, FS * (c + 1))
        in_maps.append({
            "xt": np.ascontiguousarray(src_b[:, sl].T),
            "yt": np.ascontiguousarray(tgt_b[:, sl].T),
            "w1": np.ascontiguousarray(w1_b[sl, :]),
            "cw1": np.ascontiguousarray(cw1_b[sl, :]),
            **smalls,
        })
    return in_maps


def kernel(source_features, target_features, label,
           dW1, db1, dW2, db2, dW3, db3,
           cW1, cb1, cW2, cb2, cW3, cb3,
           _want_results=False):
    from concourse.bass_utils import run_bass_kernel_spmd

    nc = _get_program()
    in_maps = _prep_inputs(source_features, target_features,
                           dW1, db1, dW2, db2, dW3, db3,
                           cW1, cb1, cW2, cb2, cW3, cb3)
    res = run_bass_kernel_spmd(nc, in_maps, core_ids=list(range(NCORES)))
    results = res.results

    # ---------------- host epilogue ----------------
    preds = np.concatenate(
        [results[c]["predsT"].T for c in range(NCORES)], axis=0
    ).astype(np.float32)  # [1280, 3]

    acc0 = results[0]["accs"].astype(np.float64)
    kx_sum = acc0[:, 0:R].sum()
    ky_sum = acc0[:, R:2 * R].sum()
    kxy_sum = 0.0
    disc_total = 0.0
    for c in range(NCORES):
        a = results[c]["accs"].astype(np.float64)
        kxy_sum += a[:, 20].sum() + a[:, 21].sum()
        disc_total += a[0, 22] + a[1, 22]

    nl = ml = NLOC
    lmmd = (R / (nl * (nl - 1)) * kx_sum
            - 2.0 / (nl * ml) * kxy_sum
            + R / (ml * (ml - 1)) * ky_sum)
    mmd_loss = np.float32(lmmd / (R * R))

    discriminator_loss = np.float32(disc_total / (2 * N * 2))

    lab = np.asarray(label).astype(np.int64).reshape(-1)
    p64 = preds.astype(np.float64)
    pmax = p64.max(axis=1, keepdims=True)
    logp = p64 - pmax - np.log(np.exp(p64 - pmax).sum(axis=1, keepdims=True))
    class_loss = -logp[np.arange(N), lab].mean()
    l2 = sum(
        (np.asarray(t, np.float64) ** 2).sum()
        for t in (cW1, cb1, cW2, cb2, cW3, cb3)
    )
    classifier_loss = np.float32(class_loss + L2_LAMBDA * l2)
    loss = np.float32(np.float32(discriminator_loss) + np.float32(classifier_loss))

    out = (preds, classifier_loss, discriminator_loss, loss, mmd_loss)
    if _want_results:
        return out, res
    return out


if __name__ == "__main__":
    rng = np.random.default_rng(0)
    inp = {
        "source_features": rng.standard_normal((N, F), dtype=np.float32),
        "target_features": rng.standard_normal((N, F), dtype=np.float32),
        "label": rng.integers(0, 3, N).astype(np.int32),
        "dW1": 0.02 * rng.standard_normal((F, 128), dtype=np.float32),
        "db1": 0.02 * rng.standard_normal(128).astype(np.float32),
        "dW2": 0.02 * rng.standard_normal((128, 64), dtype=np.float32),
        "db2": 0.02 * rng.standard_normal(64).astype(np.float32),
        "dW3": 0.02 * rng.standard_normal((64, 2), dtype=np.float32),
        "db3": 0.02 * rng.standard_normal(2).astype(np.float32),
        "cW1": 0.02 * rng.standard_normal((F, 128), dtype=np.float32),
        "cb1": 0.02 * rng.standard_normal(128).astype(np.float32),
        "cW2": 0.02 * rng.standard_normal((128, 32), dtype=np.float32),
        "cb2": 0.02 * rng.standard_normal(32).astype(np.float32),
        "cW3": 0.02 * rng.standard_normal((32, 3), dtype=np.float32),
        "cb3": 0.02 * rng.standard_normal(3).astype(np.float32),
    }
    outs = kernel(**inp)
    for o in outs:
        print(np.asarray(o).shape, np.asarray(o).reshape(-1)[:4])
